# revision 11
# baseline (speedup 1.0000x reference)
"""Weighted 2D cross-entropy (BCE-over-classes) loss on 8 Trainium2 cores.

Math (matches the reference):
  t in [0,19); pos = t>0, neg = t==0 (all pixels are pos or neg; mask == 1)
  S(i) = sum_c bce(i,c) = -[ A(i) + B(i) ]
     A(i)   = sum_c log(1-p_c(i))
     B(i)   = log(p_t(i)) - log(1-p_t(i)) = Ln(exp(-L_sel(i)) - 1)
  loss = ( (NEG/TOT)*S_pos_sum + (POS/TOT)*S_neg_sum ) / (TOT*C)

Per-core (core k <- batch element k, pure data parallel), pixel grid
[128, 4096]. Full-grid instructions amortize per-op engine overheads;
only class 18 and the tail run on quarters so the drain chain is short.
  per class: 2MB DMA; ACT L_c = Ln(1-p_c) f32->bf16 (accum_out -> U);
  DVE eq_c = (T==c) bf16 (4x), masked = eq*L (2x), A += L (bf16, 2x);
  PE identity-matmuls accumulate L_sel = sum_c masked into PSUM f32.
  tail per quarter: pos*A (TTR), expn = exp(-L_sel), B = Ln(expn-1)
  (accum_out -> sum B), pos*B (TTR).
Host folds the per-partition [128, 40] stats in float64.
"""

from contextlib import ExitStack

import numpy as np

import concourse.bass as bass
import concourse.mybir as mybir
import concourse.tile as tile
from concourse import bacc
from concourse.bass_utils import run_bass_kernel_spmd

# problem shape (hardcoded per harness contract)
N, C, H, W = 8, 19, 512, 1024
PIX = H * W          # 524288 pixels per core
P = 128              # partitions
FCOLS = PIX // P     # 4096 free columns when pixels laid out [128, 4096]
QT = FCOLS // 4      # 1024-wide quarters (class 18 + tail)
N_CORES = 8

DT = mybir.dt

# stats column layout (all f32)
COL_U = 0            # 22 cols: sum L_c per class (c18 split in quarters)
COL_POSA = 22        # 4 cols: sum pos*A per quarter
COL_POSB = 26        # 4 cols: sum pos*B per quarter
COL_SUMB = 30        # 4 cols: sum B per quarter
COL_CNT = 34         # 1 col: pos count
NCOLS = 40           # padded


def build_kernel() -> bass.Bass:
    # Bacc (not raw Bass): its compile() pipeline runs
    # generate_event_semaphores, which splits multi-sem waits to satisfy the
    # 1-wait-per-instruction TRN2 sync structs.
    nc = bacc.Bacc("TRN2")

    predict = nc.declare_dram_parameter("predict", [C, PIX], DT.float32, isOutput=False)
    target = nc.declare_dram_parameter("target", [P, FCOLS], DT.int32, isOutput=False)
    idn = nc.declare_dram_parameter("idn", [P, P], DT.bfloat16, isOutput=False)
    out = nc.declare_dram_parameter("out", [P, NCOLS], DT.float32, isOutput=True)

    pred_r = predict.rearrange("c (p f) -> c p f", p=P)  # [19, 128, 4096]

    with tile.TileContext(nc) as tc, ExitStack() as ctx:
        const = ctx.enter_context(tc.tile_pool(name="const", bufs=1))
        p_pool = ctx.enter_context(tc.tile_pool(name="p", bufs=4))
        lm_pool = ctx.enter_context(tc.tile_pool(name="lm", bufs=3))
        eq_pool = ctx.enter_context(tc.tile_pool(name="eq", bufs=2))
        msk_pool = ctx.enter_context(tc.tile_pool(name="msk", bufs=2))
        tail_pool = ctx.enter_context(tc.tile_pool(name="tail", bufs=2))
        psum_pool = ctx.enter_context(tc.tile_pool(name="ps", bufs=1, space="PSUM"))

        idn_sb = const.tile([P, P], DT.bfloat16, tag="idn")
        nc.sync.dma_start(out=idn_sb[:], in_=idn[:])

        stats = const.tile([P, NCOLS], DT.float32, tag="stats")
        nc.vector.memset(stats[:], 0.0)

        # bias=-1.0 has no pre-registered const AP; build one
        neg1 = const.tile([P, 1], DT.float32, tag="neg1")
        nc.vector.memset(neg1[:], -1.0)

        t_i32 = const.tile([P, FCOLS], DT.int32, tag="ti")
        nc.sync.dma_start(out=t_i32[:], in_=target[:])
        t_bf = const.tile([P, FCOLS], DT.bfloat16, tag="tb")
        nc.vector.tensor_copy(out=t_bf[:], in_=t_i32[:])

        # pos count (same form as the proven baseline tail op)
        cnt_scr = const.tile([P, FCOLS], DT.bfloat16, tag="cntscr")
        nc.vector.tensor_scalar(
            out=cnt_scr[:],
            in0=t_bf[:],
            scalar1=0.5,
            scalar2=None,
            op0=mybir.AluOpType.is_gt,
            op1=mybir.AluOpType.add,
            accum_out=stats[:, COL_CNT : COL_CNT + 1],
        )

        # per-pixel A accumulator (bf16, ping-pong: no in-place ops) and the
        # L_sel PSUM accumulator
        a_pool = ctx.enter_context(tc.tile_pool(name="apool", bufs=2))
        a_prev = const.tile([P, FCOLS], DT.bfloat16, tag="asb0")
        nc.vector.memset(a_prev[:], 0.0)
        lsel_ps = psum_pool.tile([P, FCOLS], DT.float32, tag="lsel")

        a_final = [None]

        def do_class(c, fsl, ucol):
            """One class over the column slice fsl; accumulate into PSUM."""
            nonlocal a_prev
            cols = fsl.stop - fsl.start
            p_t = p_pool.tile([P, cols], DT.float32, tag="p")
            nc.sync.dma_start(out=p_t[:], in_=pred_r[c, :, fsl])

            lm = lm_pool.tile([P, cols], DT.bfloat16, tag="lm")
            nc.scalar.activation(
                out=lm[:],
                in_=p_t[:],
                func=mybir.ActivationFunctionType.Ln,
                bias=1.0,
                scale=-1.0,
                accum_out=stats[:, ucol : ucol + 1],
            )

            eq = eq_pool.tile([P, cols], DT.bfloat16, tag="eq")
            nc.vector.tensor_scalar(
                out=eq[:],
                in0=t_bf[:, fsl],
                scalar1=float(c),
                scalar2=None,
                op0=mybir.AluOpType.is_equal,
            )
            msk = msk_pool.tile([P, cols], DT.bfloat16, tag="msk")
            nc.vector.tensor_mul(out=msk[:], in0=eq[:], in1=lm[:])

            for s in range(cols // 512):
                ssl = slice(fsl.start + s * 512, fsl.start + (s + 1) * 512)
                msl = slice(s * 512, (s + 1) * 512)
                nc.tensor.matmul(
                    lsel_ps[:, ssl],
                    lhsT=idn_sb[:],
                    rhs=msk[:, msl],
                    start=(c == 0),
                    stop=(c == C - 1),
                )
            return lm

        # classes 0..17 full-grid (A accumulated via ping-pong adds)
        for c in range(C - 1):
            lm = do_class(c, slice(0, FCOLS), COL_U + c)
            a_new = a_pool.tile([P, FCOLS], DT.bfloat16, tag="a")
            nc.vector.tensor_add(out=a_new[:], in0=a_prev[:], in1=lm[:])
            a_prev = a_new
        # class 18 quartered so the tail can start per quarter as soon as
        # its PSUM accumulation stops; final A add also per quarter
        a_sb = const.tile([P, FCOLS], DT.bfloat16, tag="asbF")
        for q in range(4):
            fsl = slice(q * QT, (q + 1) * QT)
            lm = do_class(C - 1, fsl, COL_U + 18 + q)
            nc.vector.tensor_add(out=a_sb[:, fsl], in0=a_prev[:, fsl], in1=lm[:])

        # ---- tail, per quarter ----
        for q in range(4):
            qsl = slice(q * QT, (q + 1) * QT)
            # sum pos*A (independent of the exp/ln chain)
            scr = tail_pool.tile([P, QT], DT.bfloat16, tag="scr")
            nc.vector.scalar_tensor_tensor(
                out=scr[:],
                in0=t_bf[:, qsl],
                scalar=0.5,
                in1=a_sb[:, qsl],
                op0=mybir.AluOpType.is_gt,
                op1=mybir.AluOpType.mult,
                accum_out=stats[:, COL_POSA + q : COL_POSA + q + 1],
            )
            # B = Ln(exp(-L_sel) - 1); accum_out on the Ln -> sum B
            expn = tail_pool.tile([P, QT], DT.float32, tag="expn")
            nc.scalar.activation(
                out=expn[:],
                in_=lsel_ps[:, qsl],
                func=mybir.ActivationFunctionType.Exp,
                scale=-1.0,
            )
            b_t = tail_pool.tile([P, QT], DT.bfloat16, tag="b")
            nc.scalar.activation(
                out=b_t[:],
                in_=expn[:],
                func=mybir.ActivationFunctionType.Ln,
                bias=neg1[:],
                accum_out=stats[:, COL_SUMB + q : COL_SUMB + q + 1],
            )
            scrb = tail_pool.tile([P, QT], DT.bfloat16, tag="scrb")
            nc.vector.scalar_tensor_tensor(
                out=scrb[:],
                in0=t_bf[:, qsl],
                scalar=0.5,
                in1=b_t[:],
                op0=mybir.AluOpType.is_gt,
                op1=mybir.AluOpType.mult,
                accum_out=stats[:, COL_POSB + q : COL_POSB + q + 1],
            )

        nc.sync.dma_start(out=out[:], in_=stats[:])

    if not nc.is_finalized():
        nc.finalize()

    return nc


def combine(outs) -> np.float32:
    """Fold the 8 cores' [128, 40] stats tiles into the scalar loss."""
    tot = np.float64(0.0)
    s_all = np.float64(0.0)
    s_pos = np.float64(0.0)
    pos = np.float64(0.0)
    for st in outs:
        st = st.astype(np.float64)
        u_all = st[:, COL_U : COL_U + 22].sum()
        pos_a = st[:, COL_POSA : COL_POSA + 4].sum()
        pos_b = st[:, COL_POSB : COL_POSB + 4].sum()
        sum_b = st[:, COL_SUMB : COL_SUMB + 4].sum()
        cnt = st[:, COL_CNT : COL_CNT + 1].sum()
        s_all += -(sum_b + u_all)
        s_pos += -(pos_b + pos_a)
        pos += cnt
        tot += PIX
    neg = tot - pos
    s_neg = s_all - s_pos
    loss = ((neg / tot) * s_pos + (pos / tot) * s_neg) / (tot * C)
    return np.float32(loss)


_NC_CACHE = None


def kernel(predict: np.ndarray, target: np.ndarray) -> np.ndarray:
    global _NC_CACHE
    if _NC_CACHE is None:
        _NC_CACHE = build_kernel()
    nc = _NC_CACHE

    import ml_dtypes

    predict = np.ascontiguousarray(predict, dtype=np.float32)
    target = np.ascontiguousarray(target, dtype=np.int32)
    idn = np.eye(P, dtype=np.float32).astype(ml_dtypes.bfloat16)

    in_maps = []
    for k in range(N_CORES):
        in_maps.append(
            {
                "predict": predict[k].reshape(C, PIX),
                "target": target[k].reshape(P, FCOLS),
                "idn": idn,
            }
        )

    res = run_bass_kernel_spmd(nc, in_maps, list(range(N_CORES)))
    return combine([res.results[k]["out"] for k in range(N_CORES)])
